# revision 17
# baseline (speedup 1.0000x reference)
"""MACAttention (windowed attention w/ persistent memory) on 8 TRN2 cores, v3.

Data-parallel over the 16 (batch, window) blocks: 2 windows per core.
All matmul operands bf16 (fp32 PSUM accumulation); tolerance is 2e-2 so
bf16's ~4e-3 worst-case path error is fine, and it halves DMA + SBUF.

v3 vs v2 (PE instruction-mix roofline war — trace showed PE issuing at
the warm streaming rate with LDWEIGHTS fully hidden, so wins come from
removing PE rows and head/tail latency):
  - sumsq on DVE pair-tree + gpsimd partition_all_reduce instead of 32
    ones-matmuls (PE -6.8us).
  - softmax denominator via gpsimd partition_all_reduce of usum (with
    the persistent-token exp folded into usum by a [16,C] DVE add)
    instead of 2 ones-matmuls per (head, window) (PE -13.6us); the
    all-reduce result is already partition-broadcast, killing the
    gpsimd broadcast + scalar copy in the old chain.
  - head: x-chunk DMAs issue first, split across the sync+gpsimd
    queues; the first wv quarter-DMA is hoisted so the v-pass matmuls
    start as soon as the first x chunks land (~12us vs 23us).
  - DMA issue cost (~600ns/issue on the issuing engine) moved off the
    scalar queue entirely (scalar = ACT only); wv streamed as one
    merged [128,8,C] DMA per (ovb,hf) instead of 8.
  - wo tiles prefetched (bufs=3) before the attention epilogue so the
    out-projection matmuls start without a DMA gap.
  - out DMA per (ec, w) right after each eviction to shorten the tail.
"""
import sys

if "/opt/trn_rl_repo" not in sys.path:
    sys.path.insert(0, "/opt/trn_rl_repo")

import numpy as np
import ml_dtypes
import concourse.bass as bass
import concourse.bass_isa as bass_isa
from concourse import bacc
import concourse.mybir as mybir
import concourse.tile as tile
from concourse.bass_utils import run_bass_kernel_spmd

F32 = mybir.dt.float32
BF16 = mybir.dt.bfloat16
AF = mybir.ActivationFunctionType
RADD = bass_isa.ReduceOp.add

HEADS = 16
DH = 128
D = 2048
C = 512          # window width (q len)
NP = 16          # persistent tokens
NCORES = 8
NW = 2           # windows per core
T = NW * C       # tokens per core
DC = 16          # d-chunks (2048/128)
SCALE = DH ** -0.5
EPS = 1e-6
THETA = 10000.0

_PERM = np.concatenate([np.arange(0, DH, 2), np.arange(1, DH, 2)])  # evens|odds

BF = ml_dtypes.bfloat16


def _build():
    nc = bacc.Bacc("TRN2", target_bir_lowering=False, debug=False)

    xT = nc.declare_dram_parameter("xT", [DC, 128, T], BF16, isOutput=False)
    wqk = nc.declare_dram_parameter("wqk", [32, 128, DC, 128], BF16, isOutput=False)
    wv = nc.declare_dram_parameter("wv", [4, 2, 128, 8, C], BF16, isOutput=False)
    wo = nc.declare_dram_parameter("wo", [16, 128, 16, 128], BF16, isOutput=False)
    cw_d = nc.declare_dram_parameter("cw_d", [128, T], BF16, isOutput=False)
    sw_d = nc.declare_dram_parameter("sw_d", [128, T], BF16, isOutput=False)
    tri_d = nc.declare_dram_parameter("tri_d", [128, 2, 128], BF16, isOutput=False)
    pmk_d = nc.declare_dram_parameter("pmk_d", [128, HEADS, NP], BF16, isOutput=False)
    pmv_d = nc.declare_dram_parameter("pmv_d", [128, HEADS, DH], BF16, isOutput=False)
    out = nc.declare_dram_parameter("out", [16, 128, NW, C], F32, isOutput=True)
    rscr = nc.dram_tensor("rscr", [1, T], F32)

    from contextlib import ExitStack

    with tile.TileContext(nc) as tc, ExitStack() as es:
        def pool(name, **kw):
            return es.enter_context(tc.tile_pool(name=name, bufs=1, **kw))

        stat = pool("stat")
        xp = pool("xp")
        vsb = pool("vsb")
        tabp = pool("tabp")
        smallp = pool("smallp")
        wqp = pool("wqp")
        pqp = pool("pqp")
        tmpp = pool("tmpp")
        qkp = pool("qkp")
        u0p = pool("u0p")
        unp = pool("unp")
        aop = pool("aop")
        wvp = pool("wvp")
        x2p = pool("x2p")
        sqp = pool("sqp")
        cwp = pool("cwp")
        wop = pool("wop")
        otp = pool("otp")
        ps = pool("ps", space="PSUM")
        if True:
            # ---- x load: first on both queues so compute starts early ----
            xt = xp.tile([128, DC, T], BF16, tag="xt")
            # first v-weight quarter first so the v-pass can start right away
            wvh00 = wvp.tile([128, 8, C], BF16, tag="wv", bufs=2, name="wv0_0")
            nc.sync.dma_start(wvh00, wv[0, 0])
            for dc in (0, 2, 4, 6, 8, 10, 12, 14):
                nc.sync.dma_start(xt[:, dc, :], xT[dc])
            for dc in (1, 3, 5, 7, 9, 11, 13, 15):
                nc.gpsimd.dma_start(xt[:, dc, :], xT[dc])

            # ---- static tiles (gpsimd queue; small) ----
            tri = stat.tile([128, 2, 128], BF16)
            nc.gpsimd.dma_start(tri, tri_d[:, :, :])
            pmk = stat.tile([128, HEADS, NP], BF16)
            nc.gpsimd.dma_start(pmk, pmk_d[:, :, :])
            pmv = stat.tile([128, HEADS, DH], BF16)
            nc.gpsimd.dma_start(pmv, pmv_d[:, :, :])
            cw = cwp.tile([128, T], BF16, tag="cw")
            nc.gpsimd.dma_start(cw, cw_d[:, :])
            sw_ = cwp.tile([128, T], BF16, tag="sw")
            nc.gpsimd.dma_start(sw_, sw_d[:, :])
            zb = stat.tile([128, 1], F32)
            nc.vector.memset(zb, 0.0)
            epst = stat.tile([128, 1], F32)
            nc.vector.memset(epst, EPS)
            ones = stat.tile([128, 1], BF16)
            nc.vector.memset(ones, 1.0)

            # ---- sumsq -> r  (two DVE accumulators, then 4 ones-matmuls) ----
            accs = [None, None]
            for i in range(8):
                xa = xt[:, 2 * i, :]
                xb = xt[:, 2 * i + 1, :]
                x2a = x2p.tile([128, T], BF16, tag="x2", bufs=1, name=f"x2a{i}")
                nc.vector.tensor_mul(x2a, xa, xa)
                x2b = x2p.tile([128, T], BF16, tag="x2b", bufs=2, name=f"x2b{i}")
                nc.scalar.square(x2b, xb)
                a = i % 2
                if accs[a] is None:
                    accs[a] = sqp.tile([128, T], BF16, tag="s1", bufs=2, name=f"acc{a}")
                    nc.vector.tensor_add(accs[a], x2a, x2b)
                else:
                    nc.vector.tensor_add(accs[a], accs[a], x2a)
                    nc.vector.tensor_add(accs[a], accs[a], x2b)
            r_tp_box = [None]

            def emit_sumsq_tail():
                # emitted mid-v-pass so the PE queue is not headed by an
                # instruction that needs every x chunk
                ps_sum = [
                    ps.tile([128, C], F32, tag="ps", bufs=8, name=f"pssum{w}")
                    for w in range(NW)
                ]
                for j in range(2):
                    for w in range(NW):
                        nc.tensor.matmul(
                            ps_sum[w][0:1, :], ones, accs[j][:, w * C : (w + 1) * C],
                            start=(j == 0), stop=(j == 1),
                        )
                sq = cwp.tile([1, T], F32, tag="sq")
                for w in range(NW):
                    nc.scalar.activation(
                        sq[:, w * C : (w + 1) * C], ps_sum[w][0:1, :],
                        AF.Sqrt, bias=epst[0:1], scale=1.0 / D,
                    )
                rcp = cwp.tile([1, T], F32, tag="rcp")
                nc.vector.reciprocal_approx_fast(rcp, sq)
                # token-partition layout for v scaling (DRAM bounce transpose)
                nc.sync.dma_start(rscr[:, :], rcp)
                r_tp = smallp.tile([128, 8], F32, tag="rtp")
                with nc.allow_non_contiguous_dma(reason="tiny r transpose"):
                    nc.sync.dma_start(r_tp, rscr[0].rearrange("(c p) -> p c", p=128))
                r_tp_box[0] = r_tp
                # fold r into rope tables (bf16)
                rcpb16 = cwp.tile([1, T], BF16, tag="rcpb16")
                nc.vector.tensor_copy(rcpb16, rcp)
                rbc = cwp.tile([128, T], BF16, tag="rbc")
                nc.gpsimd.partition_broadcast(rbc, rcpb16)
                cosr = tabp.tile([128, T], BF16, tag="cosr")
                nc.vector.tensor_mul(cosr, cw, rbc)
                sinr = tabp.tile([128, T], BF16, tag="sinr")
                nc.vector.tensor_mul(sinr, sw_, rbc)
                return cosr, sinr

            # ---- v pass: v_sb[tch] = [128 tok, 2048 ov] bf16, scaled by r ----
            # wv streamed one ovb quarter at a time (merged [128,8,C] DMAs)
            v_tiles = [
                vsb.tile([128, D], BF16, tag="v", bufs=8, name=f"v{tch}") for tch in range(8)
            ]
            cosr = sinr = None
            for ovb in range(4):
                psv = [None] * 8
                for hf in range(2):
                    if ovb == 0 and hf == 0:
                        wvh = wvh00
                    else:
                        wvh = wvp.tile(
                            [128, 8, C], BF16, tag="wv", bufs=2, name=f"wv{ovb}_{hf}"
                        )
                        nc.sync.dma_start(wvh, wv[ovb, hf])
                    for tch in range(8):
                        if ovb == 0 and hf == 0 and tch == 4:
                            # mid v-pass: 4 psv banks live + 2 sumsq banks
                            cosr, sinr = emit_sumsq_tail()
                        if hf == 0:
                            psv[tch] = ps.tile(
                                [128, C], F32, tag="ps", bufs=8,
                                name=f"psv{tch}_{ovb}",
                            )
                        for j in range(8):
                            nc.tensor.matmul(
                                psv[tch],
                                xt[:, 8 * hf + j, tch * 128 : (tch + 1) * 128],
                                wvh[:, j, :],
                                start=(hf == 0 and j == 0),
                                stop=(hf == 1 and j == 7),
                            )
                        if hf == 1:
                            nc.scalar.activation(
                                v_tiles[tch][:, ovb * C : (ovb + 1) * C], psv[tch],
                                AF.Copy, scale=r_tp_box[0][:, tch : tch + 1],
                            )

            # ---- qk projection + rope + attention, pipelined per head ----
            u0_ring = [u0p.tile([NP, C], BF16, tag="u0", bufs=2, name=f"u0r{i}") for i in range(2)]
            u0_ctr = [0]
            qk_tiles = [None] * 32
            ao_t = [[None] * HEADS for _ in range(NW)]

            def qkv_oc_start(oc):
                wt = wqp.tile([128, DC, 128], BF16, tag="wq", bufs=2, name=f"wq{oc}")
                nc.sync.dma_start(wt, wqk[oc])
                pqsb = pqp.tile([128, T], BF16, tag="pq", bufs=2, name=f"pq{oc}")
                return wt, pqsb

            def qkv_oc_window(oc, wt, pqsb, w):
                pq = ps.tile([128, C], F32, tag="ps", bufs=8, name=f"psq{oc}_{w}")
                for dc in range(DC):
                    nc.tensor.matmul(
                        pq, wt[:, dc, :], xt[:, dc, w * C : (w + 1) * C],
                        start=(dc == 0), stop=(dc == DC - 1),
                    )
                nc.scalar.activation(pqsb[:, w * C : (w + 1) * C], pq, AF.Copy)

            def qkv_oc_finish(oc, pqsb):
                # rope: qt = pqsb*cosr + swap_halves(pqsb)*sinr   (all bf16)
                # sinr is partition-rolled by 64 host-side: sinr[64:128] holds
                # -sin*r (even-row factors), sinr[0:64] holds +sin*r.
                tmp = tmpp.tile([128, T], BF16, tag="tmp", bufs=2, name=f"rt{oc}")
                nc.vector.tensor_mul(tmp[0:64], pqsb[64:128], sinr[64:128])
                nc.vector.tensor_mul(tmp[64:128], pqsb[0:64], sinr[0:64])
                qt = qkp.tile([128, T], BF16, tag="qk", bufs=6, name=f"qk{oc}")
                nc.vector.tensor_mul(qt, pqsb, cosr)
                nc.vector.tensor_add(qt, qt, tmp)
                qk_tiles[oc] = qt

            def attn_qk(h, w):
                q = qk_tiles[2 * h][:, w * C : (w + 1) * C]
                k = qk_tiles[2 * h + 1]
                # scores + exp, chunked; exact causal ranges
                s0 = ps.tile([128, C], F32, tag="ps", bufs=8, name=f"s0_{h}_{w}")
                nc.tensor.matmul(s0[0:NP, :], pmk[:, h, :], q, start=True, stop=True)
                u0 = u0_ring[u0_ctr[0] % 2]
                u0_ctr[0] += 1
                nc.scalar.activation(u0, s0[0:NP, :], AF.Exp, bias=zb[0:NP], scale=SCALE)
                un = [None] * 5
                un[0] = u0
                for cch in range(1, 5):
                    cs = 128 * (cch - 1)
                    sc = ps.tile([128, C], F32, tag="ps", bufs=8, name=f"sc{h}_{w}_{cch}")
                    nc.tensor.matmul(
                        sc[:, cs:C],
                        k[:, w * C + cs : w * C + cs + 128],
                        q[:, cs:C],
                        start=True, stop=True,
                    )
                    uc = unp.tile([128, C], BF16, tag="un", bufs=8, name=f"un{h}_{w}_{cch}")
                    nc.scalar.activation(uc[:, cs:C], sc[:, cs:C], AF.Exp, bias=zb, scale=SCALE)
                    nc.vector.tensor_mul(
                        uc[:, cs : cs + 128],
                        uc[:, cs : cs + 128],
                        tri[:, 1 if cch > 1 else 0, :],
                    )
                    un[cch] = uc
                return u0, un

            def attn_avden(h, w, u0, un):
                # denominator: DVE partial-sum of chunks (persistent rows
                # folded in), then one gpsimd all-reduce (result is already
                # broadcast across partitions)
                usum = unp.tile([128, C], BF16, tag="usum", bufs=2, name=f"us{h}_{w}")
                nc.vector.tensor_copy(usum, un[1])
                nc.vector.tensor_add(usum[0:NP, :], usum[0:NP, :], u0)
                nc.vector.tensor_add(usum[:, 128:C], usum[:, 128:C], un[2][:, 128:C])
                nc.vector.tensor_add(usum[:, 256:C], usum[:, 256:C], un[3][:, 256:C])
                nc.vector.tensor_add(usum[:, 384:C], usum[:, 384:C], un[4][:, 384:C])
                dsum = smallp.tile([128, C], F32, tag="dsum", bufs=2, name=f"ds{h}_{w}")
                nc.gpsimd.partition_all_reduce(dsum, usum, 128, RADD)
                nc.vector.reciprocal_approx_fast(dsum, dsum)
                rcb = smallp.tile([128, C], BF16, tag="rcb", bufs=2, name=f"rcb{h}_{w}")
                nc.vector.tensor_copy(rcb, dsum)
                # attn @ v (out^T accumulation)
                av = ps.tile([128, C], F32, tag="ps", bufs=8, name=f"av{h}_{w}")
                nc.tensor.matmul(av, pmv[0:NP, h, :], u0, start=True, stop=False)
                for cch in range(1, 5):
                    cs = 128 * (cch - 1)
                    nc.tensor.matmul(
                        av[:, cs:C],
                        v_tiles[4 * w + cch - 1][:, h * DH : (h + 1) * DH],
                        un[cch][:, cs:C],
                        start=False, stop=(cch == 4),
                    )
                ao = aop.tile([128, C], BF16, tag="ao", bufs=33, name=f"ao{h}_{w}")
                nc.scalar.activation(ao, av, AF.Copy)
                nc.vector.tensor_mul(ao, ao, rcb)
                ao_t[w][h] = ao

            attn_state = {}

            def attn_part(step, h):
                # step 0..3 within head h's qkv emission; operates on head h-1
                if h < 1:
                    return
                if step == 0:
                    attn_state[0] = attn_qk(h - 1, 0)
                elif step == 1:
                    attn_avden(h - 1, 0, *attn_state.pop(0))
                elif step == 2:
                    attn_state[1] = attn_qk(h - 1, 1)
                else:
                    attn_avden(h - 1, 1, *attn_state.pop(1))

            for h in range(HEADS):
                wt_q, pq_q = qkv_oc_start(2 * h)
                qkv_oc_window(2 * h, wt_q, pq_q, 0)
                attn_part(0, h)
                qkv_oc_window(2 * h, wt_q, pq_q, 1)
                qkv_oc_finish(2 * h, pq_q)
                attn_part(1, h)
                wt_k, pq_k = qkv_oc_start(2 * h + 1)
                qkv_oc_window(2 * h + 1, wt_k, pq_k, 0)
                attn_part(2, h)
                qkv_oc_window(2 * h + 1, wt_k, pq_k, 1)
                qkv_oc_finish(2 * h + 1, pq_k)
                attn_part(3, h)

            # prefetch the first out-proj weight tiles before the attention
            # epilogue so out-proj matmuls start without a DMA gap.
            # w outer so the 16 w0 groups (which only need window-0 aos) run
            # while the final head's window-1 attention drains; wo re-fetched
            # per (w, ec) pair (ring of 2).
            wot_tiles = {}

            def wo_fetch(ec):
                wot = wop.tile(
                    [128, 16, 128], BF16, tag="wo", bufs=2, name=f"wo{ec}"
                )
                nc.gpsimd.dma_start(wot, wo[ec])
                wot_tiles[ec] = wot

            for ec in range(2):
                wo_fetch(ec)

            ep0 = attn_qk(HEADS - 1, 0)
            ep1 = attn_qk(HEADS - 1, 1)
            attn_avden(HEADS - 1, 0, *ep0)
            attn_avden(HEADS - 1, 1, *ep1)

            # ---- output projection ----
            for ec in range(16):
                wot = wot_tiles[ec]
                for w in range(NW):
                    po = ps.tile([128, C], F32, tag="ps", bufs=8, name=f"po{ec}_{w}")
                    for hd in range(16):
                        nc.tensor.matmul(
                            po, wot[:, hd, :], ao_t[w][hd],
                            start=(hd == 0), stop=(hd == 15),
                        )
                    if w == 0 and ec + 2 < 16:
                        wo_fetch(ec + 2)
                    ot = otp.tile([128, C], F32, tag="ot", bufs=2, name=f"ot{ec}_{w}")
                    nc.scalar.activation(ot, po, AF.Copy)
                    nc.gpsimd.dma_start(out[ec][:, w, :], ot)
    nc.compile()
    return nc


_NC_CACHE = None


def _get_nc():
    global _NC_CACHE
    if _NC_CACHE is None:
        _NC_CACHE = _build()
    return _NC_CACHE


def _host_prep(x, norm_w, w_qkv, w_out, pm):
    xf = np.ascontiguousarray(np.asarray(x, np.float32))
    wq = np.asarray(w_qkv, np.float32) * np.asarray(norm_w, np.float32)[None, :]
    wof = np.asarray(w_out, np.float32)
    pmf = np.asarray(pm, np.float32)

    # wqk tiles [32, 128, 16, 128]; oc=2h -> q head h, oc=2h+1 -> k head h
    wqk_heads = wq[: 2 * D].reshape(2, HEADS, DH, D)[:, :, _PERM, :]  # [s,h,dh,d]
    wqk_t = np.empty((32, 128, DC, 128), np.float32)
    for h in range(HEADS):
        for s in range(2):
            blk = wqk_heads[s, h]  # [dh(o)=128, d=2048]
            wqk_t[2 * h + s] = blk.T.reshape(DC, 128, 128).transpose(1, 0, 2)

    # wv tiles [4, 16, 128, 512]: (ovb, dc, p, o) = w_v[ovb*512+o, dc*128+p]
    wv_m = wq[2 * D :]  # [2048 ov, 2048 d]
    wv_t = wv_m.reshape(4, C, DC, 128).transpose(0, 2, 3, 1)  # [4, dc, p, c]
    wv_t = np.ascontiguousarray(
        wv_t.reshape(4, 2, 8, 128, C).transpose(0, 1, 3, 2, 4)
    )  # [4, hf, p, j, c]

    # wo tiles [16, 128, 16, 128]: (ec, p, hdc, e) = wo[ec*128+e, hdc*128+p]
    wo_t = np.ascontiguousarray(wof.reshape(16, 128, 16, 128).transpose(0, 3, 2, 1))

    inv = THETA ** (-np.arange(0, DH, 2, dtype=np.float64) / DH)  # [64]

    # diagonal masks [128, 2, 128]: idx0 chunk-1 (longterm rows all-valid), idx1 plain
    kr = np.arange(128)[:, None]
    qq = np.arange(128)[None, :]
    tri_plain = (qq >= kr).astype(np.float32)
    tri_c1 = tri_plain.copy()
    tri_c1[0:16, :] = 1.0
    tri_t = np.ascontiguousarray(np.stack([tri_c1, tri_plain], axis=1))

    pmk_t = np.ascontiguousarray(pmf[0][:, :, _PERM].transpose(2, 0, 1))  # [128,h,16]
    pmv_t = np.zeros((128, HEADS, DH), np.float32)
    pmv_t[0:NP] = pmf[1].transpose(1, 0, 2)  # [16 tok, h, 128 dh]

    shared = {
        "wqk": wqk_t.astype(BF),
        "wv": wv_t.astype(BF),
        "wo": wo_t.astype(BF),
        "tri_d": tri_t.astype(BF),
        "pmk_d": pmk_t.astype(BF),
        "pmv_d": pmv_t.astype(BF),
    }

    in_maps = []
    for c in range(NCORES):
        b, tok0 = c // 4, (c % 4) * T
        xs = xf[b, tok0 : tok0 + T]  # [1024, 2048]
        # xT [dc, p, t] = x[t, dc*128+p]
        xT_c = np.ascontiguousarray(
            xs.reshape(T, DC, 128).transpose(1, 2, 0)
        ).astype(BF)
        pos = tok0 + np.arange(T, dtype=np.float64)
        ang = pos[:, None] * inv[None, :]  # [T, 64]
        cosv = np.cos(ang).astype(np.float32).T  # [64, T]
        sinv = np.sin(ang).astype(np.float32).T
        cos_c = np.concatenate([cosv, cosv], axis=0)  # [128, T]
        sin_c = np.concatenate([sinv, -sinv], axis=0)  # rolled by 64 partitions
        m = dict(shared)
        m["xT"] = xT_c
        m["cw_d"] = np.ascontiguousarray(cos_c).astype(BF)
        m["sw_d"] = np.ascontiguousarray(sin_c).astype(BF)
        in_maps.append(m)
    return in_maps


def kernel(x, norm_w, w_qkv, w_out, pm, _trace=False):
    nc = _get_nc()
    in_maps = _host_prep(x, norm_w, w_qkv, w_out, pm)
    res = run_bass_kernel_spmd(nc, in_maps, core_ids=list(range(NCORES)), trace=_trace)
    b, n = np.asarray(x).shape[0], np.asarray(x).shape[1]
    out_full = np.empty((b, n, D), np.float32)
    for c in range(NCORES):
        arr = res.results[c]["out"]  # [16, 128, NW, C]
        bb, tok0 = c // 4, (c % 4) * T
        out_full[bb, tok0 : tok0 + T] = arr.transpose(2, 3, 0, 1).reshape(T, D)
    kernel._last_results = res
    return out_full


# revision 19
# speedup vs baseline: 1.0543x; 1.0543x over previous
"""MACAttention (windowed attention w/ persistent memory) on 8 TRN2 cores, v3.

Data-parallel over the 16 (batch, window) blocks: 2 windows per core.
All matmul operands bf16 (fp32 PSUM accumulation); tolerance is 2e-2 so
bf16's ~4e-3 worst-case path error is fine, and it halves DMA + SBUF.

v3 vs v2 (PE instruction-mix roofline war — trace showed PE issuing at
the warm streaming rate with LDWEIGHTS fully hidden, so wins come from
removing PE rows and head/tail latency):
  - sumsq on DVE pair-tree + gpsimd partition_all_reduce instead of 32
    ones-matmuls (PE -6.8us).
  - softmax denominator via gpsimd partition_all_reduce of usum (with
    the persistent-token exp folded into usum by a [16,C] DVE add)
    instead of 2 ones-matmuls per (head, window) (PE -13.6us); the
    all-reduce result is already partition-broadcast, killing the
    gpsimd broadcast + scalar copy in the old chain.
  - head: x-chunk DMAs issue first, split across the sync+gpsimd
    queues; the first wv quarter-DMA is hoisted so the v-pass matmuls
    start as soon as the first x chunks land (~12us vs 23us).
  - DMA issue cost (~600ns/issue on the issuing engine) moved off the
    scalar queue entirely (scalar = ACT only); wv streamed as one
    merged [128,8,C] DMA per (ovb,hf) instead of 8.
  - wo tiles prefetched (bufs=3) before the attention epilogue so the
    out-projection matmuls start without a DMA gap.
  - out DMA per (ec, w) right after each eviction to shorten the tail.
"""
import sys

if "/opt/trn_rl_repo" not in sys.path:
    sys.path.insert(0, "/opt/trn_rl_repo")

import numpy as np
import ml_dtypes
import concourse.bass as bass
import concourse.bass_isa as bass_isa
from concourse import bacc
import concourse.mybir as mybir
import concourse.tile as tile
from concourse.bass_utils import run_bass_kernel_spmd

F32 = mybir.dt.float32
BF16 = mybir.dt.bfloat16
AF = mybir.ActivationFunctionType
RADD = bass_isa.ReduceOp.add

HEADS = 16
DH = 128
D = 2048
C = 512          # window width (q len)
NP = 16          # persistent tokens
NCORES = 8
NW = 2           # windows per core
T = NW * C       # tokens per core
DC = 16          # d-chunks (2048/128)
SCALE = DH ** -0.5
EPS = 1e-6
THETA = 10000.0

_PERM = np.concatenate([np.arange(0, DH, 2), np.arange(1, DH, 2)])  # evens|odds

BF = ml_dtypes.bfloat16


def _build():
    nc = bacc.Bacc("TRN2", target_bir_lowering=False, debug=False)

    xT = nc.declare_dram_parameter("xT", [DC, 128, T], BF16, isOutput=False)
    wqk = nc.declare_dram_parameter("wqk", [32, 128, DC, 128], BF16, isOutput=False)
    wv = nc.declare_dram_parameter("wv", [4, 2, 128, 8, C], BF16, isOutput=False)
    wo = nc.declare_dram_parameter("wo", [16, 128, 16, 128], BF16, isOutput=False)
    cw_d = nc.declare_dram_parameter("cw_d", [128, T], BF16, isOutput=False)
    sw_d = nc.declare_dram_parameter("sw_d", [128, T], BF16, isOutput=False)
    tri_d = nc.declare_dram_parameter("tri_d", [128, 2, 128], BF16, isOutput=False)
    pmk_d = nc.declare_dram_parameter("pmk_d", [128, HEADS, NP], BF16, isOutput=False)
    pmv_d = nc.declare_dram_parameter("pmv_d", [128, HEADS, DH], BF16, isOutput=False)
    out = nc.declare_dram_parameter("out", [16, 128, NW, C], F32, isOutput=True)
    rscr = nc.dram_tensor("rscr", [1, T], F32)

    from contextlib import ExitStack

    with tile.TileContext(nc) as tc, ExitStack() as es:
        def pool(name, **kw):
            return es.enter_context(tc.tile_pool(name=name, bufs=1, **kw))

        stat = pool("stat")
        xp = pool("xp")
        vsb = pool("vsb")
        tabp = pool("tabp")
        smallp = pool("smallp")
        wqp = pool("wqp")
        pqp = pool("pqp")
        tmpp = pool("tmpp")
        qkp = pool("qkp")
        u0p = pool("u0p")
        unp = pool("unp")
        aop = pool("aop")
        wvp = pool("wvp")
        x2p = pool("x2p")
        sqp = pool("sqp")
        cwp = pool("cwp")
        wop = pool("wop")
        otp = pool("otp")
        ps = pool("ps", space="PSUM")
        if True:
            # ---- x load: first on both queues so compute starts early ----
            xt = xp.tile([128, DC, T], BF16, tag="xt")
            # first v-weight quarter first so the v-pass can start right away
            wvh00 = wvp.tile([128, 8, C], BF16, tag="wv", bufs=2, name="wv0_0")
            nc.sync.dma_start(wvh00, wv[0, 0])
            for dc in (0, 2, 4, 6, 8, 10, 12, 14):
                nc.sync.dma_start(xt[:, dc, :], xT[dc])
            for dc in (1, 3, 5, 7, 9, 11, 13, 15):
                nc.gpsimd.dma_start(xt[:, dc, :], xT[dc])

            # ---- static tiles (gpsimd queue; small) ----
            tri = stat.tile([128, 2, 128], BF16)
            nc.gpsimd.dma_start(tri, tri_d[:, :, :])
            pmk = stat.tile([128, HEADS, NP], BF16)
            nc.gpsimd.dma_start(pmk, pmk_d[:, :, :])
            pmv = stat.tile([128, HEADS, DH], BF16)
            nc.gpsimd.dma_start(pmv, pmv_d[:, :, :])
            cw = cwp.tile([128, T], BF16, tag="cw")
            nc.gpsimd.dma_start(cw, cw_d[:, :])
            sw_ = cwp.tile([128, T], BF16, tag="sw")
            nc.gpsimd.dma_start(sw_, sw_d[:, :])
            zb = stat.tile([128, 1], F32)
            nc.vector.memset(zb, 0.0)
            epst = stat.tile([128, 1], F32)
            nc.vector.memset(epst, EPS)
            ones = stat.tile([128, 1], BF16)
            nc.vector.memset(ones, 1.0)

            # ---- sumsq -> r  (two DVE accumulators, then 4 ones-matmuls) ----
            accs = [None, None]
            for i in range(8):
                xa = xt[:, 2 * i, :]
                xb = xt[:, 2 * i + 1, :]
                x2a = x2p.tile([128, T], BF16, tag="x2", bufs=1, name=f"x2a{i}")
                nc.vector.tensor_mul(x2a, xa, xa)
                x2b = x2p.tile([128, T], BF16, tag="x2b", bufs=1, name=f"x2b{i}")
                nc.scalar.square(x2b, xb)
                a = i % 2
                if accs[a] is None:
                    accs[a] = sqp.tile([128, T], BF16, tag="s1", bufs=2, name=f"acc{a}")
                    nc.vector.tensor_add(accs[a], x2a, x2b)
                else:
                    nc.vector.tensor_add(accs[a], accs[a], x2a)
                    nc.vector.tensor_add(accs[a], accs[a], x2b)
            r_tp_box = [None]

            def emit_sumsq_tail():
                # emitted mid-v-pass so the PE queue is not headed by an
                # instruction that needs every x chunk
                ps_sum = [
                    ps.tile([128, C], F32, tag="ps", bufs=8, name=f"pssum{w}")
                    for w in range(NW)
                ]
                for j in range(2):
                    for w in range(NW):
                        nc.tensor.matmul(
                            ps_sum[w][0:1, :], ones, accs[j][:, w * C : (w + 1) * C],
                            start=(j == 0), stop=(j == 1),
                        )
                sq = cwp.tile([1, T], F32, tag="sq")
                for w in range(NW):
                    nc.scalar.activation(
                        sq[:, w * C : (w + 1) * C], ps_sum[w][0:1, :],
                        AF.Sqrt, bias=epst[0:1], scale=1.0 / D,
                    )
                rcp = cwp.tile([1, T], F32, tag="rcp")
                nc.vector.reciprocal_approx_fast(rcp, sq)
                # token-partition layout for v scaling (DRAM bounce transpose)
                nc.gpsimd.dma_start(rscr[:, :], rcp)
                r_tp = smallp.tile([128, 8], F32, tag="rtp")
                with nc.allow_non_contiguous_dma(reason="tiny r transpose"):
                    nc.gpsimd.dma_start(r_tp, rscr[0].rearrange("(c p) -> p c", p=128))
                r_tp_box[0] = r_tp
                # fold r into rope tables (bf16)
                rcpb16 = cwp.tile([1, T], BF16, tag="rcpb16")
                nc.vector.tensor_copy(rcpb16, rcp)
                rbc = cwp.tile([128, T], BF16, tag="rbc")
                nc.gpsimd.partition_broadcast(rbc, rcpb16)
                cosr = tabp.tile([128, T], BF16, tag="cosr")
                nc.vector.tensor_mul(cosr, cw, rbc)
                sinr = tabp.tile([128, T], BF16, tag="sinr")
                nc.vector.tensor_mul(sinr, sw_, rbc)
                return cosr, sinr

            # ---- v pass: v_sb[tch] = [128 tok, 2048 ov] bf16, scaled by r ----
            # wv streamed one ovb quarter at a time (merged [128,8,C] DMAs)
            v_tiles = [
                vsb.tile([128, D], BF16, tag="v", bufs=8, name=f"v{tch}") for tch in range(8)
            ]
            cosr = sinr = None
            for ovb in range(4):
                psv = [None] * 8
                for hf in range(2):
                    if ovb == 0 and hf == 0:
                        wvh = wvh00
                    else:
                        wvh = wvp.tile(
                            [128, 8, C], BF16, tag="wv", bufs=2, name=f"wv{ovb}_{hf}"
                        )
                        nc.sync.dma_start(wvh, wv[ovb, hf])
                    for tch in range(8):
                        if ovb == 0 and hf == 0 and tch == 4:
                            # mid v-pass: 4 psv banks live + 2 sumsq banks
                            cosr, sinr = emit_sumsq_tail()
                        if hf == 0:
                            psv[tch] = ps.tile(
                                [128, C], F32, tag="ps", bufs=8,
                                name=f"psv{tch}_{ovb}",
                            )
                        for j in range(8):
                            nc.tensor.matmul(
                                psv[tch],
                                xt[:, 8 * hf + j, tch * 128 : (tch + 1) * 128],
                                wvh[:, j, :],
                                start=(hf == 0 and j == 0),
                                stop=(hf == 1 and j == 7),
                            )
                        if hf == 1:
                            nc.scalar.activation(
                                v_tiles[tch][:, ovb * C : (ovb + 1) * C], psv[tch],
                                AF.Copy, scale=r_tp_box[0][:, tch : tch + 1],
                            )

            # ---- qk projection + rope + attention, pipelined per head ----
            u0_ring = [u0p.tile([NP, C], BF16, tag="u0", bufs=2, name=f"u0r{i}") for i in range(2)]
            u0_ctr = [0]
            qk_tiles = [None] * 32
            ao_t = [[None] * HEADS for _ in range(NW)]

            def qkv_oc_start(oc):
                wt = wqp.tile([128, DC, 128], BF16, tag="wq", bufs=2, name=f"wq{oc}")
                nc.sync.dma_start(wt, wqk[oc])
                pqsb = pqp.tile([128, T], BF16, tag="pq", bufs=2, name=f"pq{oc}")
                return wt, pqsb

            def qkv_oc_window(oc, wt, pqsb, w):
                pq = ps.tile([128, C], F32, tag="ps", bufs=8, name=f"psq{oc}_{w}")
                for dc in range(DC):
                    nc.tensor.matmul(
                        pq, wt[:, dc, :], xt[:, dc, w * C : (w + 1) * C],
                        start=(dc == 0), stop=(dc == DC - 1),
                    )
                nc.scalar.activation(pqsb[:, w * C : (w + 1) * C], pq, AF.Copy)

            def qkv_oc_finish(oc, pqsb):
                # rope: qt = pqsb*cosr + swap_halves(pqsb)*sinr   (all bf16)
                # sinr is partition-rolled by 64 host-side: sinr[64:128] holds
                # -sin*r (even-row factors), sinr[0:64] holds +sin*r.
                tmp = tmpp.tile([128, T], BF16, tag="tmp", bufs=2, name=f"rt{oc}")
                nc.vector.tensor_mul(tmp[0:64], pqsb[64:128], sinr[64:128])
                nc.vector.tensor_mul(tmp[64:128], pqsb[0:64], sinr[0:64])
                qt = qkp.tile([128, T], BF16, tag="qk", bufs=6, name=f"qk{oc}")
                nc.vector.tensor_mul(qt, pqsb, cosr)
                nc.vector.tensor_add(qt, qt, tmp)
                qk_tiles[oc] = qt

            def attn_qk(h, w):
                q = qk_tiles[2 * h][:, w * C : (w + 1) * C]
                k = qk_tiles[2 * h + 1]
                # scores + exp, chunked; exact causal ranges
                s0 = ps.tile([128, C], F32, tag="ps", bufs=8, name=f"s0_{h}_{w}")
                nc.tensor.matmul(s0[0:NP, :], pmk[:, h, :], q, start=True, stop=True)
                u0 = u0_ring[u0_ctr[0] % 2]
                u0_ctr[0] += 1
                nc.scalar.activation(u0, s0[0:NP, :], AF.Exp, bias=zb[0:NP], scale=SCALE)
                un = [None] * 5
                un[0] = u0
                for cch in range(1, 5):
                    cs = 128 * (cch - 1)
                    sc = ps.tile([128, C], F32, tag="ps", bufs=8, name=f"sc{h}_{w}_{cch}")
                    nc.tensor.matmul(
                        sc[:, cs:C],
                        k[:, w * C + cs : w * C + cs + 128],
                        q[:, cs:C],
                        start=True, stop=True,
                    )
                    uc = unp.tile([128, C], BF16, tag="un", bufs=8, name=f"un{h}_{w}_{cch}")
                    nc.scalar.activation(uc[:, cs:C], sc[:, cs:C], AF.Exp, bias=zb, scale=SCALE)
                    nc.vector.tensor_mul(
                        uc[:, cs : cs + 128],
                        uc[:, cs : cs + 128],
                        tri[:, 1 if cch > 1 else 0, :],
                    )
                    un[cch] = uc
                return u0, un

            def attn_avden(h, w, u0, un):
                # denominator: DVE partial-sum of chunks (persistent rows
                # folded in), then one gpsimd all-reduce (result is already
                # broadcast across partitions)
                usum = unp.tile([128, C], BF16, tag="usum", bufs=2, name=f"us{h}_{w}")
                nc.vector.tensor_copy(usum, un[1])
                nc.vector.tensor_add(usum[0:NP, :], usum[0:NP, :], u0)
                nc.vector.tensor_add(usum[:, 128:C], usum[:, 128:C], un[2][:, 128:C])
                nc.vector.tensor_add(usum[:, 256:C], usum[:, 256:C], un[3][:, 256:C])
                nc.vector.tensor_add(usum[:, 384:C], usum[:, 384:C], un[4][:, 384:C])
                # single ones-matmul denominator (persistent rows already in
                # usum); keeping this on the PE avoids a slow gpsimd
                # all-reduce whose latency head-of-line-blocks the vector
                # queue in front of the next head's rope ops
                dps = ps.tile([128, C], F32, tag="ps", bufs=8, name=f"dps{h}_{w}")
                nc.tensor.matmul(dps[0:1, :], ones, usum, start=True, stop=True)
                dsb = smallp.tile([1, C], F32, tag="dsb", bufs=2, name=f"dsb{h}_{w}")
                nc.scalar.activation(dsb, dps[0:1, :], AF.Copy)
                nc.vector.reciprocal_approx_fast(dsb, dsb)
                rcb1 = smallp.tile([1, C], BF16, tag="rcb1", bufs=2, name=f"rc1{h}_{w}")
                nc.vector.tensor_copy(rcb1, dsb)
                rcb = smallp.tile([128, C], BF16, tag="rcb", bufs=2, name=f"rcb{h}_{w}")
                nc.gpsimd.partition_broadcast(rcb, rcb1)
                # attn @ v (out^T accumulation)
                av = ps.tile([128, C], F32, tag="ps", bufs=8, name=f"av{h}_{w}")
                nc.tensor.matmul(av, pmv[0:NP, h, :], u0, start=True, stop=False)
                for cch in range(1, 5):
                    cs = 128 * (cch - 1)
                    nc.tensor.matmul(
                        av[:, cs:C],
                        v_tiles[4 * w + cch - 1][:, h * DH : (h + 1) * DH],
                        un[cch][:, cs:C],
                        start=False, stop=(cch == 4),
                    )
                ao = aop.tile([128, C], BF16, tag="ao", bufs=33, name=f"ao{h}_{w}")
                nc.scalar.activation(ao, av, AF.Copy)
                nc.vector.tensor_mul(ao, ao, rcb)
                ao_t[w][h] = ao

            attn_state = {}

            def attn_part(step, h):
                # step 0..3 within head h's qkv emission; operates on head h-1
                if h < 1:
                    return
                if step == 0:
                    attn_state[0] = attn_qk(h - 1, 0)
                elif step == 1:
                    attn_avden(h - 1, 0, *attn_state.pop(0))
                elif step == 2:
                    attn_state[1] = attn_qk(h - 1, 1)
                else:
                    attn_avden(h - 1, 1, *attn_state.pop(1))

            for h in range(HEADS):
                wt_q, pq_q = qkv_oc_start(2 * h)
                qkv_oc_window(2 * h, wt_q, pq_q, 0)
                attn_part(0, h)
                qkv_oc_window(2 * h, wt_q, pq_q, 1)
                qkv_oc_finish(2 * h, pq_q)
                attn_part(1, h)
                wt_k, pq_k = qkv_oc_start(2 * h + 1)
                qkv_oc_window(2 * h + 1, wt_k, pq_k, 0)
                attn_part(2, h)
                qkv_oc_window(2 * h + 1, wt_k, pq_k, 1)
                qkv_oc_finish(2 * h + 1, pq_k)
                attn_part(3, h)

            # prefetch the first out-proj weight tiles before the attention
            # epilogue so out-proj matmuls start without a DMA gap.
            # w outer so the 16 w0 groups (which only need window-0 aos) run
            # while the final head's window-1 attention drains; wo re-fetched
            # per (w, ec) pair (ring of 2).
            wot_tiles = {}

            def wo_fetch(ec):
                wot = wop.tile(
                    [128, 16, 128], BF16, tag="wo", bufs=2, name=f"wo{ec}"
                )
                nc.gpsimd.dma_start(wot, wo[ec])
                wot_tiles[ec] = wot

            for ec in range(2):
                wo_fetch(ec)

            ep0 = attn_qk(HEADS - 1, 0)
            ep1 = attn_qk(HEADS - 1, 1)
            attn_avden(HEADS - 1, 0, *ep0)
            attn_avden(HEADS - 1, 1, *ep1)

            # ---- output projection ----
            for ec in range(16):
                wot = wot_tiles[ec]
                for w in range(NW):
                    po = ps.tile([128, C], F32, tag="ps", bufs=8, name=f"po{ec}_{w}")
                    for hd in range(16):
                        nc.tensor.matmul(
                            po, wot[:, hd, :], ao_t[w][hd],
                            start=(hd == 0), stop=(hd == 15),
                        )
                    if w == 0 and ec + 2 < 16:
                        wo_fetch(ec + 2)
                    ot = otp.tile([128, C], F32, tag="ot", bufs=2, name=f"ot{ec}_{w}")
                    nc.scalar.activation(ot, po, AF.Copy)
                    nc.gpsimd.dma_start(out[ec][:, w, :], ot)
    nc.compile()
    return nc


_NC_CACHE = None


def _get_nc():
    global _NC_CACHE
    if _NC_CACHE is None:
        _NC_CACHE = _build()
    return _NC_CACHE


def _host_prep(x, norm_w, w_qkv, w_out, pm):
    xf = np.ascontiguousarray(np.asarray(x, np.float32))
    wq = np.asarray(w_qkv, np.float32) * np.asarray(norm_w, np.float32)[None, :]
    wof = np.asarray(w_out, np.float32)
    pmf = np.asarray(pm, np.float32)

    # wqk tiles [32, 128, 16, 128]; oc=2h -> q head h, oc=2h+1 -> k head h
    wqk_heads = wq[: 2 * D].reshape(2, HEADS, DH, D)[:, :, _PERM, :]  # [s,h,dh,d]
    wqk_t = np.empty((32, 128, DC, 128), np.float32)
    for h in range(HEADS):
        for s in range(2):
            blk = wqk_heads[s, h]  # [dh(o)=128, d=2048]
            wqk_t[2 * h + s] = blk.T.reshape(DC, 128, 128).transpose(1, 0, 2)

    # wv tiles [4, 16, 128, 512]: (ovb, dc, p, o) = w_v[ovb*512+o, dc*128+p]
    wv_m = wq[2 * D :]  # [2048 ov, 2048 d]
    wv_t = wv_m.reshape(4, C, DC, 128).transpose(0, 2, 3, 1)  # [4, dc, p, c]
    wv_t = np.ascontiguousarray(
        wv_t.reshape(4, 2, 8, 128, C).transpose(0, 1, 3, 2, 4)
    )  # [4, hf, p, j, c]

    # wo tiles [16, 128, 16, 128]: (ec, p, hdc, e) = wo[ec*128+e, hdc*128+p]
    wo_t = np.ascontiguousarray(wof.reshape(16, 128, 16, 128).transpose(0, 3, 2, 1))

    inv = THETA ** (-np.arange(0, DH, 2, dtype=np.float64) / DH)  # [64]

    # diagonal masks [128, 2, 128]: idx0 chunk-1 (longterm rows all-valid), idx1 plain
    kr = np.arange(128)[:, None]
    qq = np.arange(128)[None, :]
    tri_plain = (qq >= kr).astype(np.float32)
    tri_c1 = tri_plain.copy()
    tri_c1[0:16, :] = 1.0
    tri_t = np.ascontiguousarray(np.stack([tri_c1, tri_plain], axis=1))

    pmk_t = np.ascontiguousarray(pmf[0][:, :, _PERM].transpose(2, 0, 1))  # [128,h,16]
    pmv_t = np.zeros((128, HEADS, DH), np.float32)
    pmv_t[0:NP] = pmf[1].transpose(1, 0, 2)  # [16 tok, h, 128 dh]

    shared = {
        "wqk": wqk_t.astype(BF),
        "wv": wv_t.astype(BF),
        "wo": wo_t.astype(BF),
        "tri_d": tri_t.astype(BF),
        "pmk_d": pmk_t.astype(BF),
        "pmv_d": pmv_t.astype(BF),
    }

    in_maps = []
    for c in range(NCORES):
        b, tok0 = c // 4, (c % 4) * T
        xs = xf[b, tok0 : tok0 + T]  # [1024, 2048]
        # xT [dc, p, t] = x[t, dc*128+p]
        xT_c = np.ascontiguousarray(
            xs.reshape(T, DC, 128).transpose(1, 2, 0)
        ).astype(BF)
        pos = tok0 + np.arange(T, dtype=np.float64)
        ang = pos[:, None] * inv[None, :]  # [T, 64]
        cosv = np.cos(ang).astype(np.float32).T  # [64, T]
        sinv = np.sin(ang).astype(np.float32).T
        cos_c = np.concatenate([cosv, cosv], axis=0)  # [128, T]
        sin_c = np.concatenate([sinv, -sinv], axis=0)  # rolled by 64 partitions
        m = dict(shared)
        m["xT"] = xT_c
        m["cw_d"] = np.ascontiguousarray(cos_c).astype(BF)
        m["sw_d"] = np.ascontiguousarray(sin_c).astype(BF)
        in_maps.append(m)
    return in_maps


def kernel(x, norm_w, w_qkv, w_out, pm, _trace=False):
    nc = _get_nc()
    in_maps = _host_prep(x, norm_w, w_qkv, w_out, pm)
    res = run_bass_kernel_spmd(nc, in_maps, core_ids=list(range(NCORES)), trace=_trace)
    b, n = np.asarray(x).shape[0], np.asarray(x).shape[1]
    out_full = np.empty((b, n, D), np.float32)
    for c in range(NCORES):
        arr = res.results[c]["out"]  # [16, 128, NW, C]
        bb, tok0 = c // 4, (c % 4) * T
        out_full[bb, tok0 : tok0 + T] = arr.transpose(2, 3, 0, 1).reshape(T, D)
    kernel._last_results = res
    return out_full


# revision 22
# speedup vs baseline: 1.0718x; 1.0166x over previous
"""MACAttention (windowed attention w/ persistent memory) on 8 TRN2 cores, v3.

Data-parallel over the 16 (batch, window) blocks: 2 windows per core.
All matmul operands bf16 (fp32 PSUM accumulation); tolerance is 2e-2 so
bf16's ~4e-3 worst-case path error is fine, and it halves DMA + SBUF.

v3 vs v2 (PE instruction-mix roofline war — trace showed PE issuing at
the warm streaming rate with LDWEIGHTS fully hidden, so wins come from
removing PE rows and head/tail latency):
  - sumsq on DVE pair-tree + gpsimd partition_all_reduce instead of 32
    ones-matmuls (PE -6.8us).
  - softmax denominator via gpsimd partition_all_reduce of usum (with
    the persistent-token exp folded into usum by a [16,C] DVE add)
    instead of 2 ones-matmuls per (head, window) (PE -13.6us); the
    all-reduce result is already partition-broadcast, killing the
    gpsimd broadcast + scalar copy in the old chain.
  - head: x-chunk DMAs issue first, split across the sync+gpsimd
    queues; the first wv quarter-DMA is hoisted so the v-pass matmuls
    start as soon as the first x chunks land (~12us vs 23us).
  - DMA issue cost (~600ns/issue on the issuing engine) moved off the
    scalar queue entirely (scalar = ACT only); wv streamed as one
    merged [128,8,C] DMA per (ovb,hf) instead of 8.
  - wo tiles prefetched (bufs=3) before the attention epilogue so the
    out-projection matmuls start without a DMA gap.
  - out DMA per (ec, w) right after each eviction to shorten the tail.
"""
import sys

if "/opt/trn_rl_repo" not in sys.path:
    sys.path.insert(0, "/opt/trn_rl_repo")

import numpy as np
import ml_dtypes
import concourse.bass as bass
import concourse.bass_isa as bass_isa
from concourse import bacc
import concourse.mybir as mybir
import concourse.tile as tile
from concourse.bass_utils import run_bass_kernel_spmd

F32 = mybir.dt.float32
BF16 = mybir.dt.bfloat16
AF = mybir.ActivationFunctionType
RADD = bass_isa.ReduceOp.add

HEADS = 16
DH = 128
D = 2048
C = 512          # window width (q len)
NP = 16          # persistent tokens
NCORES = 8
NW = 2           # windows per core
T = NW * C       # tokens per core
DC = 16          # d-chunks (2048/128)
SCALE = DH ** -0.5
EPS = 1e-6
THETA = 10000.0

_PERM = np.concatenate([np.arange(0, DH, 2), np.arange(1, DH, 2)])  # evens|odds

BF = ml_dtypes.bfloat16


def _build():
    nc = bacc.Bacc("TRN2", target_bir_lowering=False, debug=False)

    xT = nc.declare_dram_parameter("xT", [DC, 128, T], BF16, isOutput=False)
    wqk = nc.declare_dram_parameter("wqk", [32, 128, DC, 128], BF16, isOutput=False)
    wv = nc.declare_dram_parameter("wv", [4, 2, 128, 8, C], BF16, isOutput=False)
    wo = nc.declare_dram_parameter("wo", [16, 128, 16, 128], BF16, isOutput=False)
    cw_d = nc.declare_dram_parameter("cw_d", [128, T], BF16, isOutput=False)
    sw_d = nc.declare_dram_parameter("sw_d", [128, T], BF16, isOutput=False)
    tri_d = nc.declare_dram_parameter("tri_d", [128, 2, 128], BF16, isOutput=False)
    pmk_d = nc.declare_dram_parameter("pmk_d", [128, HEADS, NP], BF16, isOutput=False)
    pmv_d = nc.declare_dram_parameter("pmv_d", [128, HEADS, DH], BF16, isOutput=False)
    out = nc.declare_dram_parameter("out", [16, 128, NW, C], F32, isOutput=True)
    rscr = nc.dram_tensor("rscr", [1, T], F32)

    from contextlib import ExitStack

    with tile.TileContext(nc) as tc, ExitStack() as es:
        def pool(name, **kw):
            return es.enter_context(tc.tile_pool(name=name, bufs=1, **kw))

        stat = pool("stat")
        xp = pool("xp")
        vsb = pool("vsb")
        tabp = pool("tabp")
        smallp = pool("smallp")
        wqp = pool("wqp")
        pqp = pool("pqp")
        tmpp = pool("tmpp")
        qkp = pool("qkp")
        u0p = pool("u0p")
        unp = pool("unp")
        aop = pool("aop")
        wvp = pool("wvp")
        x2p = pool("x2p")
        sqp = pool("sqp")
        cwp = pool("cwp")
        wop = pool("wop")
        otp = pool("otp")
        ps = pool("ps", space="PSUM")
        if True:
            # ---- x load: first on both queues so compute starts early ----
            xt = xp.tile([128, DC, T], BF16, tag="xt")
            # first v-weight quarter first (two halves so the first v-pass
            # matmuls only wait on half a megabyte)
            wvh00 = wvp.tile([128, 8, C], BF16, tag="wv", bufs=2, name="wv0_0")
            nc.sync.dma_start(wvh00[:, 0:4, :], wv[0, 0][:, 0:4, :])
            nc.sync.dma_start(wvh00[:, 4:8, :], wv[0, 0][:, 4:8, :])
            for dc in (0, 2, 4, 6, 8, 10, 12, 14):
                nc.sync.dma_start(xt[:, dc, :], xT[dc])
            for dc in (1, 3, 5, 7, 9, 11, 13, 15):
                nc.gpsimd.dma_start(xt[:, dc, :], xT[dc])

            # ---- static tiles (gpsimd queue; small) ----
            tri = stat.tile([128, 2, 128], BF16)
            nc.gpsimd.dma_start(tri, tri_d[:, :, :])
            pmk = stat.tile([128, HEADS, NP], BF16)
            nc.gpsimd.dma_start(pmk, pmk_d[:, :, :])
            pmv = stat.tile([128, HEADS, DH], BF16)
            nc.gpsimd.dma_start(pmv, pmv_d[:, :, :])
            cw = cwp.tile([128, T], BF16, tag="cw")
            nc.gpsimd.dma_start(cw, cw_d[:, :])
            sw_ = cwp.tile([128, T], BF16, tag="sw")
            nc.gpsimd.dma_start(sw_, sw_d[:, :])
            zb = stat.tile([128, 1], F32)
            nc.vector.memset(zb, 0.0)
            epst = stat.tile([128, 1], F32)
            nc.vector.memset(epst, EPS)
            ones = stat.tile([128, 1], BF16)
            nc.vector.memset(ones, 1.0)

            # ---- sumsq -> r  (per-chunk squares, PE ones-matmul reduce) ----
            # MMs for chunks 0..7 go ahead of the v-pass in the PE queue
            # (they pace with the x DMA); chunks 8..15 are emitted mid
            # v-pass via emit_sumsq_tail so nothing at the queue head waits
            # on the last x chunk.
            ps_sum = [
                ps.tile([128, C], F32, tag="ps", bufs=8, name=f"pssum{w}")
                for w in range(NW)
            ]

            def sumsq_chunk(dc):
                for w in range(NW):
                    x2 = x2p.tile(
                        [128, C], BF16, tag="x2", bufs=4, name=f"x2_{dc}_{w}"
                    )
                    xs = xt[:, dc, w * C : (w + 1) * C]
                    if dc % 2 == 0:
                        nc.vector.tensor_mul(x2, xs, xs)
                    else:
                        nc.scalar.square(x2, xs)
                    nc.tensor.matmul(
                        ps_sum[w][0:1, :], ones, x2,
                        start=(dc == 0), stop=(dc == DC - 1),
                    )

            for dc in range(8):
                sumsq_chunk(dc)
            r_tp_box = [None]

            def emit_sumsq_tail():
                for dc in range(8, DC):
                    sumsq_chunk(dc)
                sq = cwp.tile([1, T], F32, tag="sq")
                for w in range(NW):
                    nc.scalar.activation(
                        sq[:, w * C : (w + 1) * C], ps_sum[w][0:1, :],
                        AF.Sqrt, bias=epst[0:1], scale=1.0 / D,
                    )
                rcp = cwp.tile([1, T], F32, tag="rcp")
                nc.vector.reciprocal_approx_fast(rcp, sq)
                # token-partition layout for v scaling (DRAM bounce transpose)
                nc.gpsimd.dma_start(rscr[:, :], rcp)
                r_tp = smallp.tile([128, 8], F32, tag="rtp")
                with nc.allow_non_contiguous_dma(reason="tiny r transpose"):
                    nc.gpsimd.dma_start(r_tp, rscr[0].rearrange("(c p) -> p c", p=128))
                r_tp_box[0] = r_tp
                # fold r into rope tables (bf16)
                rcpb16 = cwp.tile([1, T], BF16, tag="rcpb16")
                nc.vector.tensor_copy(rcpb16, rcp)
                rbc = cwp.tile([128, T], BF16, tag="rbc")
                nc.gpsimd.partition_broadcast(rbc, rcpb16)
                cosr = tabp.tile([128, T], BF16, tag="cosr")
                nc.vector.tensor_mul(cosr, cw, rbc)
                sinr = tabp.tile([128, T], BF16, tag="sinr")
                nc.vector.tensor_mul(sinr, sw_, rbc)
                return cosr, sinr

            # ---- v pass: v_sb[tch] = [128 tok, 2048 ov] bf16, scaled by r ----
            # wv streamed one ovb quarter at a time (merged [128,8,C] DMAs)
            v_tiles = [
                vsb.tile([128, D], BF16, tag="v", bufs=8, name=f"v{tch}") for tch in range(8)
            ]
            cosr = sinr = None
            for ovb in range(4):
                psv = [None] * 8
                for hf in range(2):
                    if ovb == 0 and hf == 0:
                        wvh = wvh00
                    else:
                        wvh = wvp.tile(
                            [128, 8, C], BF16, tag="wv", bufs=2, name=f"wv{ovb}_{hf}"
                        )
                        # alternate queues so wv streaming isn't serialized
                        # behind the x chunks on one DMA queue
                        weng = nc.gpsimd if (2 * ovb + hf) % 2 == 0 else nc.sync
                        weng.dma_start(wvh, wv[ovb, hf])
                    for tch in range(8):
                        if ovb == 0 and hf == 0 and tch == 4:
                            # mid v-pass: 4 psv banks live + 2 sumsq banks
                            cosr, sinr = emit_sumsq_tail()
                        if hf == 0:
                            psv[tch] = ps.tile(
                                [128, C], F32, tag="ps", bufs=8,
                                name=f"psv{tch}_{ovb}",
                            )
                        for j in range(8):
                            nc.tensor.matmul(
                                psv[tch],
                                xt[:, 8 * hf + j, tch * 128 : (tch + 1) * 128],
                                wvh[:, j, :],
                                start=(hf == 0 and j == 0),
                                stop=(hf == 1 and j == 7),
                            )
                        if hf == 1:
                            nc.scalar.activation(
                                v_tiles[tch][:, ovb * C : (ovb + 1) * C], psv[tch],
                                AF.Copy, scale=r_tp_box[0][:, tch : tch + 1],
                            )

            # ---- qk projection + rope + attention, pipelined per head ----
            u0_ring = [u0p.tile([NP, C], BF16, tag="u0", bufs=2, name=f"u0r{i}") for i in range(2)]
            u0_ctr = [0]
            qk_tiles = [None] * 32
            ao_t = [[None] * HEADS for _ in range(NW)]

            def qkv_oc_start(oc):
                wt = wqp.tile([128, DC, 128], BF16, tag="wq", bufs=2, name=f"wq{oc}")
                nc.sync.dma_start(wt, wqk[oc])
                pqsb = pqp.tile([128, T], BF16, tag="pq", bufs=2, name=f"pq{oc}")
                return wt, pqsb

            def qkv_oc_window(oc, wt, pqsb, w):
                pq = ps.tile([128, C], F32, tag="ps", bufs=8, name=f"psq{oc}_{w}")
                for dc in range(DC):
                    nc.tensor.matmul(
                        pq, wt[:, dc, :], xt[:, dc, w * C : (w + 1) * C],
                        start=(dc == 0), stop=(dc == DC - 1),
                    )
                nc.scalar.activation(pqsb[:, w * C : (w + 1) * C], pq, AF.Copy)

            def qkv_oc_finish(oc, pqsb):
                # rope: qt = pqsb*cosr + swap_halves(pqsb)*sinr   (all bf16)
                # sinr is partition-rolled by 64 host-side: sinr[64:128] holds
                # -sin*r (even-row factors), sinr[0:64] holds +sin*r.
                tmp = tmpp.tile([128, T], BF16, tag="tmp", bufs=2, name=f"rt{oc}")
                nc.vector.tensor_mul(tmp[0:64], pqsb[64:128], sinr[64:128])
                nc.vector.tensor_mul(tmp[64:128], pqsb[0:64], sinr[0:64])
                qt = qkp.tile([128, T], BF16, tag="qk", bufs=6, name=f"qk{oc}")
                nc.vector.tensor_mul(qt, pqsb, cosr)
                nc.vector.tensor_add(qt, qt, tmp)
                qk_tiles[oc] = qt

            def attn_qk(h, w):
                q = qk_tiles[2 * h][:, w * C : (w + 1) * C]
                k = qk_tiles[2 * h + 1]
                # scores + exp, chunked; exact causal ranges
                s0 = ps.tile([128, C], F32, tag="ps", bufs=8, name=f"s0_{h}_{w}")
                nc.tensor.matmul(s0[0:NP, :], pmk[:, h, :], q, start=True, stop=True)
                u0 = u0_ring[u0_ctr[0] % 2]
                u0_ctr[0] += 1
                nc.scalar.activation(u0, s0[0:NP, :], AF.Exp, bias=zb[0:NP], scale=SCALE)
                un = [None] * 5
                un[0] = u0
                for cch in range(1, 5):
                    cs = 128 * (cch - 1)
                    sc = ps.tile([128, C], F32, tag="ps", bufs=8, name=f"sc{h}_{w}_{cch}")
                    nc.tensor.matmul(
                        sc[:, cs:C],
                        k[:, w * C + cs : w * C + cs + 128],
                        q[:, cs:C],
                        start=True, stop=True,
                    )
                    uc = unp.tile([128, C], BF16, tag="un", bufs=8, name=f"un{h}_{w}_{cch}")
                    nc.scalar.activation(uc[:, cs:C], sc[:, cs:C], AF.Exp, bias=zb, scale=SCALE)
                    nc.vector.tensor_mul(
                        uc[:, cs : cs + 128],
                        uc[:, cs : cs + 128],
                        tri[:, 1 if cch > 1 else 0, :],
                    )
                    un[cch] = uc
                return u0, un

            def attn_avden(h, w, u0, un):
                # denominator: DVE partial-sum of chunks (persistent rows
                # folded in), then one gpsimd all-reduce (result is already
                # broadcast across partitions)
                usum = unp.tile([128, C], BF16, tag="usum", bufs=2, name=f"us{h}_{w}")
                nc.vector.tensor_copy(usum, un[1])
                nc.vector.tensor_add(usum[0:NP, :], usum[0:NP, :], u0)
                nc.vector.tensor_add(usum[:, 128:C], usum[:, 128:C], un[2][:, 128:C])
                nc.vector.tensor_add(usum[:, 256:C], usum[:, 256:C], un[3][:, 256:C])
                nc.vector.tensor_add(usum[:, 384:C], usum[:, 384:C], un[4][:, 384:C])
                # single ones-matmul denominator (persistent rows already in
                # usum); keeping this on the PE avoids a slow gpsimd
                # all-reduce whose latency head-of-line-blocks the vector
                # queue in front of the next head's rope ops
                dps = ps.tile([128, C], F32, tag="ps", bufs=8, name=f"dps{h}_{w}")
                nc.tensor.matmul(dps[0:1, :], ones, usum, start=True, stop=True)
                dsb = smallp.tile([1, C], F32, tag="dsb", bufs=2, name=f"dsb{h}_{w}")
                nc.scalar.activation(dsb, dps[0:1, :], AF.Copy)
                nc.vector.reciprocal_approx_fast(dsb, dsb)
                rcb1 = smallp.tile([1, C], BF16, tag="rcb1", bufs=2, name=f"rc1{h}_{w}")
                nc.vector.tensor_copy(rcb1, dsb)
                rcb = smallp.tile([128, C], BF16, tag="rcb", bufs=2, name=f"rcb{h}_{w}")
                nc.gpsimd.partition_broadcast(rcb, rcb1)
                # attn @ v (out^T accumulation)
                av = ps.tile([128, C], F32, tag="ps", bufs=8, name=f"av{h}_{w}")
                nc.tensor.matmul(av, pmv[0:NP, h, :], u0, start=True, stop=False)
                for cch in range(1, 5):
                    cs = 128 * (cch - 1)
                    nc.tensor.matmul(
                        av[:, cs:C],
                        v_tiles[4 * w + cch - 1][:, h * DH : (h + 1) * DH],
                        un[cch][:, cs:C],
                        start=False, stop=(cch == 4),
                    )
                ao = aop.tile([128, C], BF16, tag="ao", bufs=33, name=f"ao{h}_{w}")
                nc.scalar.activation(ao, av, AF.Copy)
                nc.vector.tensor_mul(ao, ao, rcb)
                ao_t[w][h] = ao

            attn_state = {}

            def attn_part(step, h):
                # step 0..3 within head h's qkv emission; operates on head h-1
                if h < 1:
                    return
                if step == 0:
                    attn_state[0] = attn_qk(h - 1, 0)
                elif step == 1:
                    attn_avden(h - 1, 0, *attn_state.pop(0))
                elif step == 2:
                    attn_state[1] = attn_qk(h - 1, 1)
                else:
                    attn_avden(h - 1, 1, *attn_state.pop(1))

            for h in range(HEADS):
                wt_q, pq_q = qkv_oc_start(2 * h)
                qkv_oc_window(2 * h, wt_q, pq_q, 0)
                attn_part(0, h)
                qkv_oc_window(2 * h, wt_q, pq_q, 1)
                qkv_oc_finish(2 * h, pq_q)
                attn_part(1, h)
                wt_k, pq_k = qkv_oc_start(2 * h + 1)
                qkv_oc_window(2 * h + 1, wt_k, pq_k, 0)
                attn_part(2, h)
                qkv_oc_window(2 * h + 1, wt_k, pq_k, 1)
                qkv_oc_finish(2 * h + 1, pq_k)
                attn_part(3, h)

            # prefetch the first out-proj weight tiles before the attention
            # epilogue so out-proj matmuls start without a DMA gap.
            # w outer so the 16 w0 groups (which only need window-0 aos) run
            # while the final head's window-1 attention drains; wo re-fetched
            # per (w, ec) pair (ring of 2).
            wot_tiles = {}

            def wo_fetch(ec):
                wot = wop.tile(
                    [128, 16, 128], BF16, tag="wo", bufs=2, name=f"wo{ec}"
                )
                nc.gpsimd.dma_start(wot, wo[ec])
                wot_tiles[ec] = wot

            for ec in range(2):
                wo_fetch(ec)

            ep0 = attn_qk(HEADS - 1, 0)
            ep1 = attn_qk(HEADS - 1, 1)
            attn_avden(HEADS - 1, 0, *ep0)
            attn_avden(HEADS - 1, 1, *ep1)

            # ---- output projection ----
            for ec in range(16):
                wot = wot_tiles[ec]
                for w in range(NW):
                    po = ps.tile([128, C], F32, tag="ps", bufs=8, name=f"po{ec}_{w}")
                    for hd in range(16):
                        nc.tensor.matmul(
                            po, wot[:, hd, :], ao_t[w][hd],
                            start=(hd == 0), stop=(hd == 15),
                        )
                    if w == 0 and ec + 2 < 16:
                        wo_fetch(ec + 2)
                    ot = otp.tile([128, C], F32, tag="ot", bufs=2, name=f"ot{ec}_{w}")
                    nc.scalar.activation(ot, po, AF.Copy)
                    nc.gpsimd.dma_start(out[ec][:, w, :], ot)
    nc.compile()
    return nc


_NC_CACHE = None


def _get_nc():
    global _NC_CACHE
    if _NC_CACHE is None:
        _NC_CACHE = _build()
    return _NC_CACHE


def _host_prep(x, norm_w, w_qkv, w_out, pm):
    xf = np.ascontiguousarray(np.asarray(x, np.float32))
    wq = np.asarray(w_qkv, np.float32) * np.asarray(norm_w, np.float32)[None, :]
    wof = np.asarray(w_out, np.float32)
    pmf = np.asarray(pm, np.float32)

    # wqk tiles [32, 128, 16, 128]; oc=2h -> q head h, oc=2h+1 -> k head h
    wqk_heads = wq[: 2 * D].reshape(2, HEADS, DH, D)[:, :, _PERM, :]  # [s,h,dh,d]
    wqk_t = np.empty((32, 128, DC, 128), np.float32)
    for h in range(HEADS):
        for s in range(2):
            blk = wqk_heads[s, h]  # [dh(o)=128, d=2048]
            wqk_t[2 * h + s] = blk.T.reshape(DC, 128, 128).transpose(1, 0, 2)

    # wv tiles [4, 16, 128, 512]: (ovb, dc, p, o) = w_v[ovb*512+o, dc*128+p]
    wv_m = wq[2 * D :]  # [2048 ov, 2048 d]
    wv_t = wv_m.reshape(4, C, DC, 128).transpose(0, 2, 3, 1)  # [4, dc, p, c]
    wv_t = np.ascontiguousarray(
        wv_t.reshape(4, 2, 8, 128, C).transpose(0, 1, 3, 2, 4)
    )  # [4, hf, p, j, c]

    # wo tiles [16, 128, 16, 128]: (ec, p, hdc, e) = wo[ec*128+e, hdc*128+p]
    wo_t = np.ascontiguousarray(wof.reshape(16, 128, 16, 128).transpose(0, 3, 2, 1))

    inv = THETA ** (-np.arange(0, DH, 2, dtype=np.float64) / DH)  # [64]

    # diagonal masks [128, 2, 128]: idx0 chunk-1 (longterm rows all-valid), idx1 plain
    kr = np.arange(128)[:, None]
    qq = np.arange(128)[None, :]
    tri_plain = (qq >= kr).astype(np.float32)
    tri_c1 = tri_plain.copy()
    tri_c1[0:16, :] = 1.0
    tri_t = np.ascontiguousarray(np.stack([tri_c1, tri_plain], axis=1))

    pmk_t = np.ascontiguousarray(pmf[0][:, :, _PERM].transpose(2, 0, 1))  # [128,h,16]
    pmv_t = np.zeros((128, HEADS, DH), np.float32)
    pmv_t[0:NP] = pmf[1].transpose(1, 0, 2)  # [16 tok, h, 128 dh]

    shared = {
        "wqk": wqk_t.astype(BF),
        "wv": wv_t.astype(BF),
        "wo": wo_t.astype(BF),
        "tri_d": tri_t.astype(BF),
        "pmk_d": pmk_t.astype(BF),
        "pmv_d": pmv_t.astype(BF),
    }

    in_maps = []
    for c in range(NCORES):
        b, tok0 = c // 4, (c % 4) * T
        xs = xf[b, tok0 : tok0 + T]  # [1024, 2048]
        # xT [dc, p, t] = x[t, dc*128+p]
        xT_c = np.ascontiguousarray(
            xs.reshape(T, DC, 128).transpose(1, 2, 0)
        ).astype(BF)
        pos = tok0 + np.arange(T, dtype=np.float64)
        ang = pos[:, None] * inv[None, :]  # [T, 64]
        cosv = np.cos(ang).astype(np.float32).T  # [64, T]
        sinv = np.sin(ang).astype(np.float32).T
        cos_c = np.concatenate([cosv, cosv], axis=0)  # [128, T]
        sin_c = np.concatenate([sinv, -sinv], axis=0)  # rolled by 64 partitions
        m = dict(shared)
        m["xT"] = xT_c
        m["cw_d"] = np.ascontiguousarray(cos_c).astype(BF)
        m["sw_d"] = np.ascontiguousarray(sin_c).astype(BF)
        in_maps.append(m)
    return in_maps


def kernel(x, norm_w, w_qkv, w_out, pm, _trace=False):
    nc = _get_nc()
    in_maps = _host_prep(x, norm_w, w_qkv, w_out, pm)
    res = run_bass_kernel_spmd(nc, in_maps, core_ids=list(range(NCORES)), trace=_trace)
    b, n = np.asarray(x).shape[0], np.asarray(x).shape[1]
    out_full = np.empty((b, n, D), np.float32)
    for c in range(NCORES):
        arr = res.results[c]["out"]  # [16, 128, NW, C]
        bb, tok0 = c // 4, (c % 4) * T
        out_full[bb, tok0 : tok0 + T] = arr.transpose(2, 3, 0, 1).reshape(T, D)
    kernel._last_results = res
    return out_full
